# revision 4
# baseline (speedup 1.0000x reference)
"""Trainium2 Bass kernel for nn_MobileOptimizedSimpleClawMatrix (v2).

Computation (per batch element b):
    vp  = x_v @ Wv.T + bv                     [L, D]
    lp  = x_l @ Wl.T + bl                     [L, D]
    attn = softmax(vp @ lp.T, axis=-1)
    out = attn @ vp @ Wov.T + attn.T @ lp @ Wol.T + bo
        (Wo = [Wov | Wol]; algebraically identical to
         concat([attn @ vp, attn.T @ lp], -1) @ Wo.T + bo)

Key structure vs v1:
  - Wo folded through the attention: vpo = vp @ Wov.T (lhsT = vpT, which sim
    needs anyway), lpo = lp @ Wol.T (lhsT = lpT, no transpose).  The whole
    phase-D out-GEMM, avT spill/reload and lp re-transposes disappear.
  - attn, vpo, lpo are bf16 (rel err ~3e-3 end to end); projections and the
    LxL sim GEMM stay fp32r.  All matmul moving operands are >=256 wide so
    everything runs at 1 cycle/row.
  - attn^T is produced by the DMA crossbar (dma_start_transpose, bf16),
    not the PE; vp^T uses PE transposes with a bf16 identity (1 cycle/row).
  - bv folds into the vpT transpose evacuation (ACT bias), bl into the lpT
    evacuation (ACT bias), bo via a broadcast tile + DVE add at the av
    evacuation.  No bias matmuls on the PE.
  - attn rows and av+bo rows (both bf16) spill to DRAM; phase C re-reads
    attn column slices and av rows, computes al = attn.T @ lpo and emits
    out = al + av_rows directly.
  - Weights stream on the Pool DMA queue, activations on the SP queue.

Sharding: batch B=8 across 8 cores, data parallel, params replicated.
"""

import os

os.environ.setdefault("JAX_PLATFORMS", "")

import numpy as np

B = 8
L = 2048  # tokens
D = 768  # feature dim
P = 128
NK = D // P  # 6 chunks over feature dim
NT = L // P  # 16 token blocks
NJS = L // 256  # 8 j slices for the al phase

_CACHE = {}


def _build_nc(n_reps: int = 1, dbg: bool = False):
    from contextlib import ExitStack

    import concourse.bacc as bacc
    import concourse.mybir as mybir
    import concourse.tile as tile
    from concourse.masks import make_identity

    F32 = mybir.dt.float32
    F32R = mybir.dt.float32r
    BF16 = mybir.dt.bfloat16
    Exp = mybir.ActivationFunctionType.Exp
    Identity = mybir.ActivationFunctionType.Identity
    X = mybir.AxisListType.X
    Mult = mybir.AluOpType.mult
    Add = mybir.AluOpType.add

    nc = bacc.Bacc(
        "TRN2", target_bir_lowering=False, debug=False, num_devices=B,
        num_swdge_queues=4,
    )

    # ---- DRAM I/O (per core; host pre-transposes x and W) ----
    xvT = nc.dram_tensor("xvT", [D, L], F32R, kind="ExternalInput")
    xlT = nc.dram_tensor("xlT", [D, L], F32R, kind="ExternalInput")
    wvT = nc.dram_tensor("wvT", [D, D], F32R, kind="ExternalInput")  # Wv.T [d, e]
    wlT = nc.dram_tensor("wlT", [D, D], F32R, kind="ExternalInput")  # Wl.T [d, e]
    woT = nc.dram_tensor("woT", [2 * D, D], F32R, kind="ExternalInput")  # Wo.T
    bv = nc.dram_tensor("bv", [D], F32, kind="ExternalInput")
    bl = nc.dram_tensor("bl", [D], F32, kind="ExternalInput")
    bo = nc.dram_tensor("bo", [D], F32, kind="ExternalInput")
    out = nc.dram_tensor("out", [L, D], F32, kind="ExternalOutput")

    xvT_v = xvT[:].rearrange("(k p) t -> p k t", p=P)
    xlT_v = xlT[:].rearrange("(k p) t -> p k t", p=P)
    wvT_v = wvT[:].rearrange("(k p) e -> p k e", p=P)
    wlT_v = wlT[:].rearrange("(k p) e -> p k e", p=P)
    woT_v = woT[:].rearrange("(k p) e -> p k e", p=P)

    with ExitStack() as ctx:
        tc = ctx.enter_context(tile.TileContext(nc))

        dram = ctx.enter_context(tc.tile_pool(name="dram", bufs=1, space="DRAM"))
        if dbg:
            attn_spill = nc.dram_tensor("dbg_attn", [L, L], BF16, kind="ExternalOutput")
            av_spill = nc.dram_tensor("dbg_av", [L, D], BF16, kind="ExternalOutput")
        else:
            attn_spill = dram.tile([L, L], BF16)  # normalized attn rows
            av_spill = dram.tile([L, D], BF16)  # attn @ vpo + bo rows
        attn_cols = attn_spill[:].rearrange("(c p) j -> p c j", p=P)
        av_rows = av_spill[:].rearrange("(c p) e -> p c e", p=P)

        # ---- persistent pools ----
        const = ctx.enter_context(tc.tile_pool(name="const", bufs=1))
        vpT_pool = ctx.enter_context(tc.tile_pool(name="vpT", bufs=1))
        lpT_pool = ctx.enter_context(tc.tile_pool(name="lpT", bufs=1))
        ident = const.tile([P, P], F32R)
        ident_b = const.tile([P, P], BF16)
        ones1 = const.tile([1, P], F32R)
        with ExitStack() as ictx:
            init = ictx.enter_context(tc.tile_pool(name="init", bufs=1))
            ident_f = init.tile([P, P], F32)
            make_identity(nc, ident_f[:])
            nc.vector.tensor_copy(ident[:], ident_f[:])
            nc.vector.tensor_copy(ident_b[:], ident_f[:])
            ones_f = init.tile([1, P], F32)
            nc.gpsimd.memset(ones_f[:], 1.0)
            nc.vector.tensor_copy(ones1[:], ones_f[:])

        for _rep in range(n_reps):
            # persistent per-rep arrays (vpo/lpo/wo allocated at phase B)
            vpT_t = vpT_pool.tile([P, NK, L], F32R, tag="vpT")  # vp^T (+bv)
            lpT_t = lpT_pool.tile([P, NK, L], F32R, tag="lpT")  # lp^T (+bl)

            # ============ Phase A: projections, transposes, vpo ============
            with ExitStack() as actx:
                wv_pool = actx.enter_context(tc.tile_pool(name="wv", bufs=1))
                wl_pool = actx.enter_context(tc.tile_pool(name="wl", bufs=1))
                xv_pool = actx.enter_context(tc.tile_pool(name="xv", bufs=6))
                xl_pool = actx.enter_context(tc.tile_pool(name="xl", bufs=3))
                vp_pool = actx.enter_context(tc.tile_pool(name="vp", bufs=2))
                pa_big = actx.enter_context(
                    tc.tile_pool(name="pa_big", bufs=2, space="PSUM")
                )
                pa_lp = actx.enter_context(
                    tc.tile_pool(name="pa_lp", bufs=2, space="PSUM")
                )
                pa_tr = actx.enter_context(
                    tc.tile_pool(name="pa_tr", bufs=2, space="PSUM")
                )

                wv12 = wv_pool.tile([P, NK, D], F32R, tag="wv12")
                wl12 = wl_pool.tile([P, NK, D], F32R, tag="wl12")
                # Pool queue: first x block, then WvT[0] so the PE starts ASAP
                xvb0 = xv_pool.tile([P, NK, P], F32R, tag="xvb", name="xvb0")
                nc.gpsimd.dma_start(xvb0[:], xvT_v[:, :, 0:P])
                nc.gpsimd.dma_start(wv12[:, 0, :], wvT_v[:, 0, :])
                for k in range(1, NK):
                    nc.gpsimd.dma_start(wv12[:, k, :], wvT_v[:, k, :])
                # SP queue: bias cols
                bl_col = wl_pool.tile([P, NK], F32, tag="bl_col", name="bl_col")
                nc.sync.dma_start(bl_col[:], bl[:].rearrange("(o p) -> p o", p=P))
                bv_col = wv_pool.tile([P, NK], F32, tag="bv_col", name="bv_col")
                nc.sync.dma_start(bv_col[:], bv[:].rearrange("(o p) -> p o", p=P))

                # weight loads spread over the first token blocks (Pool queue)
                wl_sched = {6: [0, 1, 2], 8: [3, 4, 5]}

                xl_tiles = {}

                def lpT_slice_load(g):
                    # Pool queue (strictly ordered after the weight loads);
                    # two half-transfers so the x_v stream interleaves finer
                    xlg = xl_pool.tile([P, NK, 512], F32R, tag="xlg")
                    xl_tiles[g] = xlg
                    nc.gpsimd.dma_start(
                        xlg[:, :, 0:256], xlT_v[:, :, g * 512 : g * 512 + 256]
                    )
                    nc.gpsimd.dma_start(
                        xlg[:, :, 256:512],
                        xlT_v[:, :, g * 512 + 256 : (g + 1) * 512],
                    )

                def lpT_slice(g):
                    xlg = xl_tiles.pop(g)
                    for me in range(NK):
                        lps = pa_lp.tile([P, 512], F32, tag="lps")
                        for k in range(NK):
                            nc.tensor.matmul(
                                lps[:], wl12[:, k, me * P : (me + 1) * P],
                                xlg[:, k, :],
                                start=(k == 0), stop=(k == NK - 1),
                            )
                        nc.scalar.activation(
                            lpT_t[:, me, g * 512 : (g + 1) * 512], lps[:],
                            Identity, bias=bl_col[:, me : me + 1], scale=1.0,
                        )

                for tb in range(NT):
                    if tb == 0:
                        xvb = xvb0
                    else:
                        xvb = xv_pool.tile([P, NK, P], F32R, tag="xvb")
                        nc.sync.dma_start(
                            xvb[:], xvT_v[:, :, tb * P : (tb + 1) * P]
                        )
                    vps = pa_big.tile([P, D], F32, tag="big")
                    for k in range(NK):
                        nc.tensor.matmul(
                            vps[:, 0:512], xvb[:, k, :], wv12[:, k, 0:512],
                            start=(k == 0), stop=(k == NK - 1),
                        )
                        nc.tensor.matmul(
                            vps[:, 512:D], xvb[:, k, :], wv12[:, k, 512:D],
                            start=(k == 0), stop=(k == NK - 1),
                        )
                    vp_tile = vp_pool.tile([P, D], F32R, tag="vpt")
                    nc.scalar.copy(vp_tile[:], vps[:])
                    # transpose to vpT with bv folded in via ACT bias
                    for k3 in range(0, NK, 3):
                        ptr = pa_tr.tile([P, 3, P], F32R, tag="ptr3")
                        for j in range(3):
                            nc.tensor.transpose(
                                ptr[:, j, :],
                                vp_tile[:, (k3 + j) * P : (k3 + j + 1) * P],
                                ident[:],
                            )
                        for j in range(3):
                            nc.scalar.activation(
                                vpT_t[:, k3 + j, tb * P : (tb + 1) * P],
                                ptr[:, j, :],
                                Identity, bias=bv_col[:, k3 + j : k3 + j + 1],
                                scale=1.0,
                            )
                    for k in wl_sched.get(tb, []):
                        nc.gpsimd.dma_start(wl12[:, k, :], wlT_v[:, k, :])
                    # 512-wide lpT slices (3 x_l buffers; the 4th load waits
                    # for the buffer slice 0 frees)
                    if tb in (9, 11, 13):
                        lpT_slice_load((tb - 9) // 2)
                    if tb == 13:
                        lpT_slice(0)
                    elif tb == 15:
                        lpT_slice_load(3)
                        lpT_slice(1)
                lpT_slice(2)
                lpT_slice(3)

            # ============ Phases B+C (lpo spans both) ============
            bcctx = ExitStack()
            lpo_pool = bcctx.enter_context(tc.tile_pool(name="lpo", bufs=1))
            lpo_t = lpo_pool.tile([P, NT, D], BF16, tag="lpo")

            # ============ Phase B: wo loads, vpo burst, attention, av, lpo ====
            with ExitStack() as bctx:
                wol_pool = bctx.enter_context(tc.tile_pool(name="wol", bufs=1))
                vpo_pool = bctx.enter_context(tc.tile_pool(name="vpo", bufs=1))
                pb_big = bctx.enter_context(
                    tc.tile_pool(name="pb_big", bufs=1, space="PSUM")
                )
                wol_t = wol_pool.tile([P, NK, D], F32R, tag="wol")
                vpo_t = vpo_pool.tile([P, NT, D], BF16, tag="vpo")

                # prefix: Wo loads, bo broadcast, vpo burst
                with ExitStack() as pctx:
                    wov_pool = pctx.enter_context(tc.tile_pool(name="wov", bufs=1))
                    wov_t = wov_pool.tile([P, NK, D], F32R, tag="wov")
                    bor = wov_pool.tile([1, D], F32R, tag="bor", name="bor")
                    nc.gpsimd.dma_start(bor[:], bo[:].unsqueeze(0))
                    for k in range(NK):
                        nc.gpsimd.dma_start(wov_t[:, k, :], woT_v[:, k, :])
                    for k in range(NK):
                        nc.gpsimd.dma_start(wol_t[:, k, :], woT_v[:, NK + k, :])
                    bo_bc = const.tile([P, D], BF16, tag="bo_bc", name="bo_bc")
                    bps = pb_big.tile([P, D], F32, tag="pbig")
                    nc.tensor.matmul(bps[:, 0:512], ones1[:], bor[:, 0:512])
                    nc.tensor.matmul(bps[:, 512:D], ones1[:], bor[:, 512:D])
                    nc.vector.tensor_copy(bo_bc[:], bps[:])
                    for tb in range(NT):
                        # vpo[tb] = vp[tb] @ Wov.T  (lhsT = vpT)
                        vpos = pb_big.tile([P, D], F32, tag="pbig")
                        for k in range(NK):
                            nc.tensor.matmul(
                                vpos[:, 0:512],
                                vpT_t[:, k, tb * P : (tb + 1) * P],
                                wov_t[:, k, 0:512],
                                start=(k == 0), stop=(k == NK - 1),
                            )
                            nc.tensor.matmul(
                                vpos[:, 512:D],
                                vpT_t[:, k, tb * P : (tb + 1) * P],
                                wov_t[:, k, 512:D],
                                start=(k == 0), stop=(k == NK - 1),
                            )
                        nc.scalar.copy(vpo_t[:, tb, :], vpos[:])

                simsb_pool = bctx.enter_context(tc.tile_pool(name="simsb", bufs=2))
                attn_pool = bctx.enter_context(tc.tile_pool(name="attn", bufs=2))
                attnT_pool = bctx.enter_context(tc.tile_pool(name="attnT", bufs=2))
                avrow_pool = bctx.enter_context(tc.tile_pool(name="avrow", bufs=2))
                stat_pool = bctx.enter_context(tc.tile_pool(name="stat", bufs=4))
                pb_sim = bctx.enter_context(
                    tc.tile_pool(name="pb_sim", bufs=2, space="PSUM")
                )
                pb_tr = bctx.enter_context(
                    tc.tile_pool(name="pb_tr", bufs=2, space="PSUM")
                )

                attn_tiles = {}
                attnT_tiles = {}

                def lpo_chunk(ib):
                    # lpo[ib] = lp[ib] @ Wol.T  (lhsT = lpT)
                    lpos = pb_big.tile([P, D], F32, tag="pbig")
                    for k in range(NK):
                        nc.tensor.matmul(
                            lpos[:, 0:512],
                            lpT_t[:, k, ib * P : (ib + 1) * P],
                            wol_t[:, k, 0:512],
                            start=(k == 0), stop=(k == NK - 1),
                        )
                        nc.tensor.matmul(
                            lpos[:, 512:D],
                            lpT_t[:, k, ib * P : (ib + 1) * P],
                            wol_t[:, k, 512:D],
                            start=(k == 0), stop=(k == NK - 1),
                        )
                    nc.scalar.copy(lpo_t[:, ib, :], lpos[:])

                def emit_attnT(k):
                    # PE transposes (bf16, 1 cycle/row) in groups of 4
                    a = attn_tiles.pop(k)
                    t = attnT_pool.tile(
                        [P, NT, P], BF16, tag="attnT", name=f"attnT_{k}"
                    )
                    attnT_tiles[k] = t
                    for g4 in range(0, NT, 4):
                        ptr = pb_tr.tile([P, 4, P], BF16, tag="ptr4")
                        for j in range(4):
                            nc.tensor.transpose(
                                ptr[:, j, :],
                                a[:, (g4 + j) * P : (g4 + j + 1) * P],
                                ident_b[:],
                            )
                        nc.scalar.copy(t[:, g4 : g4 + 4, :], ptr[:])

                def emit_av(k):
                    t = attnT_tiles.pop(k)
                    avp = pb_big.tile([P, D], F32, tag="pbig")
                    for jc in range(NT):
                        nc.tensor.matmul(
                            avp[:, 0:512], t[:, jc, :], vpo_t[:, jc, 0:512],
                            start=(jc == 0), stop=(jc == NT - 1),
                        )
                        nc.tensor.matmul(
                            avp[:, 512:D], t[:, jc, :], vpo_t[:, jc, 512:D],
                            start=(jc == 0), stop=(jc == NT - 1),
                        )
                    avrow = avrow_pool.tile([P, D], BF16, tag="avrow")
                    nc.vector.scalar_tensor_tensor(
                        avrow[:], avp[:], 1.0, bo_bc[:], op0=Mult, op1=Add
                    )
                    nc.sync.dma_start(av_rows[:, k, :], avrow[:])

                for iblk in range(NT):
                    # sim row block [128, 2048] in two psum halves
                    simsb = simsb_pool.tile([P, L], F32, tag="simsb")
                    for h in range(2):
                        simh = pb_sim.tile([P, 1024], F32, tag="simh")
                        for ns in range(2):
                            lo = h * 1024 + ns * 512
                            for k in range(NK):
                                nc.tensor.matmul(
                                    simh[:, ns * 512 : (ns + 1) * 512],
                                    vpT_t[:, k, iblk * P : (iblk + 1) * P],
                                    lpT_t[:, k, lo : lo + 512],
                                    start=(k == 0), stop=(k == NK - 1),
                                )
                            sl_sb = slice(lo, lo + 512)
                            sl_ps = slice(ns * 512, (ns + 1) * 512)
                            if ns == 0:
                                nc.scalar.copy(simsb[:, sl_sb], simh[:, sl_ps])
                            else:
                                nc.vector.tensor_copy(simsb[:, sl_sb], simh[:, sl_ps])
                    # lpo, then attnT transposes of the previous block (PE
                    # cover for the lpo evac), then av of block k-2
                    if iblk < NT - 1:
                        lpo_chunk(iblk)
                    if iblk >= 1:
                        emit_attnT(iblk - 1)
                    if iblk >= 2:
                        emit_av(iblk - 2)
                    # softmax (rows) -> bf16 attn
                    negm = stat_pool.tile([P, 1], F32, tag="negm")
                    nc.vector.reduce_max(negm[:], simsb[:], axis=X, negate=True)
                    attn = attn_pool.tile([P, L], BF16, tag="attn", name=f"attn_{iblk}")
                    attn_tiles[iblk] = attn
                    z = stat_pool.tile([P, 1], F32, tag="z")
                    nc.scalar.activation(
                        attn[:], simsb[:], Exp, bias=negm[:], scale=1.0,
                        accum_out=z[:],
                    )
                    rz = stat_pool.tile([P, 1], F32, tag="rz")
                    nc.vector.reciprocal(rz[:], z[:])
                    nc.vector.tensor_scalar_mul(attn[:, 0:1024], attn[:, 0:1024], rz[:])
                    nc.vector.tensor_scalar_mul(attn[:, 1024:L], attn[:, 1024:L], rz[:])
                    nc.sync.dma_start(
                        attn_spill[iblk * P : (iblk + 1) * P, :], attn[:]
                    )
                emit_attnT(NT - 1)
                emit_av(NT - 2)
                lpo_chunk(NT - 1)
                emit_av(NT - 1)

            # ============ Phase C: al + output ============
            with ExitStack() as cctx:
                colt_pool = cctx.enter_context(tc.tile_pool(name="colt", bufs=6))
                avr_pool = cctx.enter_context(tc.tile_pool(name="avr", bufs=3))
                outsb_pool = cctx.enter_context(tc.tile_pool(name="outsb", bufs=2))
                pc_al = cctx.enter_context(
                    tc.tile_pool(name="pc_al", bufs=2, space="PSUM")
                )

                for js in range(NJS):
                    # attn column slice in 4 chunks of 4 i-blocks each
                    chunks = []
                    for q in range(4):
                        ch = colt_pool.tile([P, 4, 256], BF16, tag="colt")
                        chunks.append(ch)
                        nc.gpsimd.dma_start(
                            ch[:],
                            attn_cols[:, 4 * q : 4 * q + 4,
                                      js * 256 : (js + 1) * 256],
                        )
                    avr = avr_pool.tile([P, 2, D], BF16, tag="avr")
                    nc.gpsimd.dma_start(
                        avr[:], av_rows[:, 2 * js : 2 * js + 2, :]
                    )
                    for half in range(2):
                        jb = 2 * js + half
                        alp = pc_al.tile([P, D], F32, tag="alp")
                        for ic in range(NT):
                            lhsT = chunks[ic // 4][:, ic % 4,
                                                   half * P : (half + 1) * P]
                            nc.tensor.matmul(
                                alp[:, 0:512], lhsT, lpo_t[:, ic, 0:512],
                                start=(ic == 0), stop=(ic == NT - 1),
                            )
                            nc.tensor.matmul(
                                alp[:, 512:D], lhsT, lpo_t[:, ic, 512:D],
                                start=(ic == 0), stop=(ic == NT - 1),
                            )
                        outsb = outsb_pool.tile([P, D], F32, tag="outsb")
                        nc.vector.scalar_tensor_tensor(
                            outsb[:], alp[:], 1.0, avr[:, half, :],
                            op0=Mult, op1=Add,
                        )
                        nc.sync.dma_start(out[jb * P : (jb + 1) * P, :], outsb[:])

            bcctx.close()

    nc.compile()
    return nc


def _build_sharded(nc):
    """Cache a jitted sharded executable so repeat calls skip retracing."""
    import jax
    import concourse.mybir as mybir
    from jax.sharding import Mesh, PartitionSpec
    from jax.experimental.shard_map import shard_map
    from concourse.bass2jax import (
        _bass_exec_p,
        install_neuronx_cc_hook,
        partition_id_tensor,
    )

    install_neuronx_cc_hook()
    partition_name = nc.partition_id_tensor.name if nc.partition_id_tensor else None
    in_names, out_names, out_avals, zero_outs = [], [], [], []
    for alloc in nc.m.functions[0].allocations:
        if not isinstance(alloc, mybir.MemoryLocationSet):
            continue
        name = alloc.memorylocations[0].name
        if alloc.kind == "ExternalInput":
            if name != partition_name:
                in_names.append(name)
        elif alloc.kind == "ExternalOutput":
            shape = tuple(alloc.tensor_shape)
            dtype = mybir.dt.np(alloc.dtype)
            out_names.append(name)
            out_avals.append(jax.core.ShapedArray(shape, dtype))
            zero_outs.append(np.zeros(shape, dtype))
    n_params = len(in_names)
    n_outs = len(out_avals)
    all_in_names = list(in_names) + list(out_names)
    if partition_name is not None:
        all_in_names.append(partition_name)
    donate = tuple(range(n_params, n_params + n_outs))

    def _body(*args):
        operands = list(args)
        if partition_name is not None:
            operands.append(partition_id_tensor())
        return tuple(
            _bass_exec_p.bind(
                *operands,
                out_avals=tuple(out_avals),
                in_names=tuple(all_in_names),
                out_names=tuple(out_names),
                lowering_input_output_aliases=(),
                sim_require_finite=True,
                sim_require_nnan=True,
                nc=nc,
            )
        )

    devices = jax.devices()[:B]
    mesh = Mesh(np.asarray(devices), ("core",))
    sharding = jax.sharding.NamedSharding(mesh, PartitionSpec("core"))
    sharded = jax.jit(
        shard_map(
            _body,
            mesh=mesh,
            in_specs=(PartitionSpec("core"),) * (n_params + n_outs),
            out_specs=(PartitionSpec("core"),) * n_outs,
            check_rep=False,
        ),
        donate_argnums=donate,
        keep_unused=True,
    )

    import jax.numpy as jnp

    zero_shapes = tuple((B * z.shape[0], *z.shape[1:]) for z in zero_outs)
    zero_dtypes = tuple(z.dtype for z in zero_outs)

    @jax.jit
    def _make_zeros():
        return tuple(jnp.zeros(s, d) for s, d in zip(zero_shapes, zero_dtypes))

    def device_zeros():
        return jax.device_put(_make_zeros(), [sharding] * len(zero_shapes))

    return {
        "sharded": sharded,
        "in_names": in_names,
        "out_names": out_names,
        "zero_outs": zero_outs,
        "out_avals": out_avals,
        "sharding": sharding,
        "device_zeros": device_zeros,
    }


def _prep_inputs(vision_features, language_features, Wv, bv, Wl, bl, Wo, bo):
    wvT = np.ascontiguousarray(np.asarray(Wv, dtype=np.float32).T)
    wlT = np.ascontiguousarray(np.asarray(Wl, dtype=np.float32).T)
    woT = np.ascontiguousarray(np.asarray(Wo, dtype=np.float32).T)
    bv = np.asarray(bv, dtype=np.float32)
    bl = np.asarray(bl, dtype=np.float32)
    bo = np.asarray(bo, dtype=np.float32)
    vision_features = np.asarray(vision_features, dtype=np.float32)
    language_features = np.asarray(language_features, dtype=np.float32)

    in_maps = []
    for b in range(B):
        in_maps.append(
            {
                "xvT": np.ascontiguousarray(vision_features[b].T),
                "xlT": np.ascontiguousarray(language_features[b].T),
                "wvT": wvT,
                "wlT": wlT,
                "woT": woT,
                "bv": bv,
                "bl": bl,
                "bo": bo,
            }
        )
    return in_maps


def kernel(
    vision_features, language_features, Wv, bv, Wl, bl, Wo, bo
) -> np.ndarray:
    from concourse.bass_utils import run_bass_kernel_spmd

    nc = _CACHE.get("nc")
    if nc is None:
        nc = _build_nc()
        _CACHE["nc"] = nc

    in_maps = _prep_inputs(
        vision_features, language_features, Wv, bv, Wl, bl, Wo, bo
    )

    try:
        ex = _CACHE.get("ex")
        if ex is None:
            ex = _build_sharded(nc)
            _CACHE["ex"] = ex
        concat_in = [
            np.concatenate([m[n] for m in in_maps], axis=0)
            for n in ex["in_names"]
        ]
        out_arrs = ex["sharded"](*concat_in, *ex["device_zeros"]())
        i = ex["out_names"].index("out")
        full = np.asarray(out_arrs[i]).reshape(B, *ex["out_avals"][i].shape)
        return full.astype(np.float32)
    except Exception:
        res = run_bass_kernel_spmd(nc, in_maps, list(range(B)))
        return np.stack([res.results[b]["out"] for b in range(B)]).astype(np.float32)


# revision 5
# speedup vs baseline: 1.2704x; 1.2704x over previous
"""Trainium2 Bass kernel for nn_MobileOptimizedSimpleClawMatrix (v3).

Fully SBUF-resident: no DRAM scratch at all.  vpT/lpT/attn/vpo/lpo are bf16
(l2 err ~9e-3 vs the 2e-2 gate); only the input projections run in fp32r.
DMA per core is just inputs (17.5 MB) + output (6.3 MB).

    out[jb] = attn[:, jb].T @ lpo  +  attnT(jb) @ vpo  + bo
computed per 128-token block in one PSUM accumulation (64 matmuls), with the
attn^T block transposed on the PE right before use.

Sharding: batch B=8 across 8 cores, data parallel, params replicated.
"""

import os

os.environ.setdefault("JAX_PLATFORMS", "")

import numpy as np

B = 8
L = 2048  # tokens
D = 768  # feature dim
P = 128
NK = D // P  # 6 chunks over feature dim
NT = L // P  # 16 token blocks

_CACHE = {}


def _build_nc(n_reps: int = 1, dbg: bool = False):
    from contextlib import ExitStack

    import concourse.bacc as bacc
    import concourse.mybir as mybir
    import concourse.tile as tile
    from concourse.masks import make_identity

    F32 = mybir.dt.float32
    F32R = mybir.dt.float32r
    BF16 = mybir.dt.bfloat16
    Exp = mybir.ActivationFunctionType.Exp
    Identity = mybir.ActivationFunctionType.Identity
    X = mybir.AxisListType.X
    Mult = mybir.AluOpType.mult
    Add = mybir.AluOpType.add

    nc = bacc.Bacc(
        "TRN2", target_bir_lowering=False, debug=False, num_devices=B,
        num_swdge_queues=4,
    )

    # ---- DRAM I/O (per core; host pre-transposes x and W) ----
    xvT = nc.dram_tensor("xvT", [D, L], F32R, kind="ExternalInput")
    xlT = nc.dram_tensor("xlT", [D, L], F32R, kind="ExternalInput")
    wvT = nc.dram_tensor("wvT", [D, D], F32R, kind="ExternalInput")  # Wv.T
    wlT = nc.dram_tensor("wlT", [D, D], F32R, kind="ExternalInput")  # Wl.T
    woT = nc.dram_tensor("woT", [2 * D, D], BF16, kind="ExternalInput")  # Wo.T
    bv = nc.dram_tensor("bv", [D], F32, kind="ExternalInput")
    bl = nc.dram_tensor("bl", [D], F32, kind="ExternalInput")
    bo = nc.dram_tensor("bo", [D], F32, kind="ExternalInput")
    out = nc.dram_tensor("out", [L, D], F32, kind="ExternalOutput")

    xvT_v = xvT[:].rearrange("(k p) t -> p k t", p=P)
    xlT_v = xlT[:].rearrange("(k p) t -> p k t", p=P)
    wvT_v = wvT[:].rearrange("(k p) e -> p k e", p=P)
    wlT_v = wlT[:].rearrange("(k p) e -> p k e", p=P)
    woT_v = woT[:].rearrange("(k p) e -> p k e", p=P)

    with ExitStack() as ctx:
        tc = ctx.enter_context(tile.TileContext(nc))

        # ---- persistent pools ----
        const = ctx.enter_context(tc.tile_pool(name="const", bufs=1))
        vpT_pool = ctx.enter_context(tc.tile_pool(name="vpT", bufs=1))
        lpT_pool = ctx.enter_context(tc.tile_pool(name="lpT", bufs=1))
        wov_pool = ctx.enter_context(tc.tile_pool(name="wov", bufs=1))
        wol_pool = ctx.enter_context(tc.tile_pool(name="wol", bufs=1))

        ident_b = const.tile([P, P], BF16)
        ones1 = const.tile([1, P], F32R)
        with ExitStack() as ictx:
            init = ictx.enter_context(tc.tile_pool(name="init", bufs=1))
            ident_f = init.tile([P, P], F32)
            make_identity(nc, ident_f[:])
            nc.vector.tensor_copy(ident_b[:], ident_f[:])
            ones_f = init.tile([1, P], F32)
            nc.gpsimd.memset(ones_f[:], 1.0)
            nc.vector.tensor_copy(ones1[:], ones_f[:])

        for _rep in range(n_reps):
            # (attn/vpo/lpo tiles allocated at phase B so phase A's working
            # pools fit; their pools are outer-scope so they live into C)
            vpT_t = vpT_pool.tile([P, NK, L], BF16, tag="vpT")  # vp^T (+bv)
            lpT_t = lpT_pool.tile([P, NK, L], BF16, tag="lpT")  # lp^T (+bl)
            wov_t = wov_pool.tile([P, NK, D], BF16, tag="wov")
            wol_t = wol_pool.tile([P, NK, D], BF16, tag="wol")

            # ============ Phase A: projections + transposes ============
            with ExitStack() as actx:
                wv_pool = actx.enter_context(tc.tile_pool(name="wv", bufs=1))
                wl_pool = actx.enter_context(tc.tile_pool(name="wl", bufs=1))
                xv_pool = actx.enter_context(tc.tile_pool(name="xv", bufs=6))
                xl_pool = actx.enter_context(tc.tile_pool(name="xl", bufs=3))
                vp_pool = actx.enter_context(tc.tile_pool(name="vp", bufs=2))
                pa_big = actx.enter_context(
                    tc.tile_pool(name="pa_big", bufs=2, space="PSUM")
                )
                pa_lp = actx.enter_context(
                    tc.tile_pool(name="pa_lp", bufs=2, space="PSUM")
                )
                pa_tr = actx.enter_context(
                    tc.tile_pool(name="pa_tr", bufs=2, space="PSUM")
                )

                wv12 = wv_pool.tile([P, NK, D], F32R, tag="wv12")
                wl12 = wl_pool.tile([P, NK, D], F32R, tag="wl12")
                # Pool queue: first x block, then WvT[0] so the PE starts ASAP
                xvb0 = xv_pool.tile([P, NK, P], F32R, tag="xvb", name="xvb0")
                nc.gpsimd.dma_start(xvb0[:], xvT_v[:, :, 0:P])
                for k in range(NK):
                    nc.gpsimd.dma_start(wv12[:, k, :], wvT_v[:, k, :])
                # SP queue: bias cols
                bl_col = wl_pool.tile([P, NK], F32, tag="bl_col", name="bl_col")
                nc.sync.dma_start(bl_col[:], bl[:].rearrange("(o p) -> p o", p=P))
                bv_col = wv_pool.tile([P, NK], F32, tag="bv_col", name="bv_col")
                nc.sync.dma_start(bv_col[:], bv[:].rearrange("(o p) -> p o", p=P))

                # weight loads spread over the token blocks (Pool queue)
                wl_sched = {6: [0, 1, 2], 8: [3, 4, 5]}
                wo_sched = {4: [0, 1, 2], 5: [3, 4, 5], 6: [6, 7, 8], 7: [9, 10, 11]}

                xl_tiles = {}

                def lpT_slice_load(g):
                    xlg = xl_pool.tile([P, NK, 512], F32R, tag="xlg")
                    xl_tiles[g] = xlg
                    nc.gpsimd.dma_start(
                        xlg[:, :, 0:256], xlT_v[:, :, g * 512 : g * 512 + 256]
                    )
                    nc.gpsimd.dma_start(
                        xlg[:, :, 256:512],
                        xlT_v[:, :, g * 512 + 256 : (g + 1) * 512],
                    )

                def lpT_slice(g):
                    xlg = xl_tiles.pop(g)
                    for me in range(NK):
                        lps = pa_lp.tile([P, 512], F32, tag="lps")
                        for k in range(NK):
                            nc.tensor.matmul(
                                lps[:], wl12[:, k, me * P : (me + 1) * P],
                                xlg[:, k, :],
                                start=(k == 0), stop=(k == NK - 1),
                            )
                        nc.scalar.activation(
                            lpT_t[:, me, g * 512 : (g + 1) * 512], lps[:],
                            Identity, bias=bl_col[:, me : me + 1], scale=1.0,
                        )

                for tb in range(NT):
                    if tb == 0:
                        xvb = xvb0
                    else:
                        xvb = xv_pool.tile([P, NK, P], F32R, tag="xvb")
                        nc.sync.dma_start(
                            xvb[:], xvT_v[:, :, tb * P : (tb + 1) * P]
                        )
                    vps = pa_big.tile([P, D], F32, tag="big")
                    for k in range(NK):
                        nc.tensor.matmul(
                            vps[:, 0:512], xvb[:, k, :], wv12[:, k, 0:512],
                            start=(k == 0), stop=(k == NK - 1),
                        )
                        nc.tensor.matmul(
                            vps[:, 512:D], xvb[:, k, :], wv12[:, k, 512:D],
                            start=(k == 0), stop=(k == NK - 1),
                        )
                    vp_tile = vp_pool.tile([P, D], BF16, tag="vpt")
                    nc.scalar.copy(vp_tile[:], vps[:])
                    # transpose to vpT with bv folded in via ACT bias
                    for k3 in range(0, NK, 3):
                        ptr = pa_tr.tile([P, 3, P], BF16, tag="ptr3")
                        for j in range(3):
                            nc.tensor.transpose(
                                ptr[:, j, :],
                                vp_tile[:, (k3 + j) * P : (k3 + j + 1) * P],
                                ident_b[:],
                            )
                        for j in range(3):
                            nc.scalar.activation(
                                vpT_t[:, k3 + j, tb * P : (tb + 1) * P],
                                ptr[:, j, :],
                                Identity, bias=bv_col[:, k3 + j : k3 + j + 1],
                                scale=1.0,
                            )
                    for k in wl_sched.get(tb, []):
                        nc.gpsimd.dma_start(wl12[:, k, :], wlT_v[:, k, :])
                    for k in wo_sched.get(tb, []):
                        dst = wov_t if k < NK else wol_t
                        nc.gpsimd.dma_start(dst[:, k % NK, :], woT_v[:, k, :])
                    # 512-wide lpT slices (3 x_l buffers; the 4th load waits
                    # for the buffer slice 0 frees)
                    if tb in (9, 11, 13):
                        lpT_slice_load((tb - 9) // 2)
                    if tb == 13:
                        lpT_slice(0)
                    elif tb == 15:
                        lpT_slice_load(3)
                        lpT_slice(1)
                lpT_slice(2)
                lpT_slice(3)

            # ============ Phases B+C (attn/vpo/lpo span both) ============
            bcctx = ExitStack()
            attn_pool = bcctx.enter_context(tc.tile_pool(name="attnt", bufs=1))
            vpo_pool = bcctx.enter_context(tc.tile_pool(name="vpo", bufs=1))
            lpo_pool = bcctx.enter_context(tc.tile_pool(name="lpo", bufs=1))
            attn_t = attn_pool.tile([P, NT, L], BF16, tag="attn")
            vpo_t = vpo_pool.tile([P, NT, D], BF16, tag="vpo")
            lpo_t = lpo_pool.tile([P, NT, D], BF16, tag="lpo")

            # ============ Phase B: attention rows, vpo, lpo ============
            with ExitStack() as bctx:
                simsb_pool = bctx.enter_context(tc.tile_pool(name="simsb", bufs=2))
                stat_pool = bctx.enter_context(tc.tile_pool(name="stat", bufs=4))
                borp = bctx.enter_context(tc.tile_pool(name="borp", bufs=1))
                pb_sim = bctx.enter_context(
                    tc.tile_pool(name="pb_sim", bufs=2, space="PSUM")
                )
                pb_big = bctx.enter_context(
                    tc.tile_pool(name="pb_big", bufs=2, space="PSUM")
                )

                # bo broadcast (used in phase C's output evacuations)
                bor = borp.tile([1, D], F32R, tag="bor", name="bor")
                nc.gpsimd.dma_start(bor[:], bo[:].unsqueeze(0))
                bo_bc = const.tile([P, D], BF16, tag="bo_bc", name="bo_bc")
                bps = pb_big.tile([P, D], F32, tag="pbig")
                nc.tensor.matmul(bps[:, 0:512], ones1[:], bor[:, 0:512])
                nc.tensor.matmul(bps[:, 512:D], ones1[:], bor[:, 512:D])
                nc.vector.tensor_copy(bo_bc[:], bps[:])

                def proj_chunk(ib, src_t, w_t, dst_t):
                    # dst[ib] = src[ib] @ W.T  (lhsT = srcT, 512+256 chains)
                    ps = pb_big.tile([P, D], F32, tag="pbig")
                    for k in range(NK):
                        nc.tensor.matmul(
                            ps[:, 0:512],
                            src_t[:, k, ib * P : (ib + 1) * P],
                            w_t[:, k, 0:512],
                            start=(k == 0), stop=(k == NK - 1),
                        )
                        nc.tensor.matmul(
                            ps[:, 512:D],
                            src_t[:, k, ib * P : (ib + 1) * P],
                            w_t[:, k, 512:D],
                            start=(k == 0), stop=(k == NK - 1),
                        )
                    nc.scalar.copy(dst_t[:, ib, :], ps[:])

                for iblk in range(NT):
                    # sim row block [128, 2048] in two psum halves
                    simsb = simsb_pool.tile([P, L], F32, tag="simsb")
                    for h in range(2):
                        simh = pb_sim.tile([P, 1024], F32, tag="simh")
                        for ns in range(2):
                            lo = h * 1024 + ns * 512
                            for k in range(NK):
                                nc.tensor.matmul(
                                    simh[:, ns * 512 : (ns + 1) * 512],
                                    vpT_t[:, k, iblk * P : (iblk + 1) * P],
                                    lpT_t[:, k, lo : lo + 512],
                                    start=(k == 0), stop=(k == NK - 1),
                                )
                            sl_sb = slice(lo, lo + 512)
                            sl_ps = slice(ns * 512, (ns + 1) * 512)
                            if ns == 0:
                                nc.scalar.copy(simsb[:, sl_sb], simh[:, sl_ps])
                            else:
                                nc.vector.tensor_copy(simsb[:, sl_sb], simh[:, sl_ps])
                    # vpo/lpo chunks for this block (PE cover for softmax)
                    proj_chunk(iblk, vpT_t, wov_t, vpo_t)
                    proj_chunk(iblk, lpT_t, wol_t, lpo_t)
                    # softmax (rows) -> bf16 attn, SBUF-resident
                    negm = stat_pool.tile([P, 1], F32, tag="negm")
                    nc.vector.reduce_max(negm[:], simsb[:], axis=X, negate=True)
                    z = stat_pool.tile([P, 1], F32, tag="z")
                    nc.scalar.activation(
                        attn_t[:, iblk, :], simsb[:], Exp, bias=negm[:],
                        scale=1.0, accum_out=z[:],
                    )
                    rz = stat_pool.tile([P, 1], F32, tag="rz")
                    nc.vector.reciprocal(rz[:], z[:])
                    nc.vector.tensor_scalar_mul(
                        attn_t[:, iblk, 0:1024], attn_t[:, iblk, 0:1024], rz[:]
                    )
                    nc.vector.tensor_scalar_mul(
                        attn_t[:, iblk, 1024:L], attn_t[:, iblk, 1024:L], rz[:]
                    )

            # ============ Phase C: out[jb] = av'(jb) + al'(jb) + bo ============
            with ExitStack() as cctx:
                attnT_pool = cctx.enter_context(tc.tile_pool(name="attnT", bufs=2))
                outsb_pool = cctx.enter_context(tc.tile_pool(name="outsb", bufs=2))
                pc_out = cctx.enter_context(
                    tc.tile_pool(name="pc_out", bufs=2, space="PSUM")
                )
                pc_tr = cctx.enter_context(
                    tc.tile_pool(name="pc_tr", bufs=2, space="PSUM")
                )

                for jb in range(NT):
                    # attn^T for this token block (PE transposes, bf16)
                    t = attnT_pool.tile([P, NT, P], BF16, tag="attnT")
                    for g4 in range(0, NT, 4):
                        ptr = pc_tr.tile([P, 4, P], BF16, tag="ptr4")
                        for j in range(4):
                            nc.tensor.transpose(
                                ptr[:, j, :],
                                attn_t[:, jb, (g4 + j) * P : (g4 + j + 1) * P],
                                ident_b[:],
                            )
                        nc.scalar.copy(t[:, g4 : g4 + 4, :], ptr[:])
                    ops = pc_out.tile([P, D], F32, tag="ops")
                    # av' chain: attnT(jb) @ vpo
                    for jc in range(NT):
                        nc.tensor.matmul(
                            ops[:, 0:512], t[:, jc, :], vpo_t[:, jc, 0:512],
                            start=(jc == 0), stop=False,
                        )
                        nc.tensor.matmul(
                            ops[:, 512:D], t[:, jc, :], vpo_t[:, jc, 512:D],
                            start=(jc == 0), stop=False,
                        )
                    # al' chain accumulates on top: attn[:, jb].T @ lpo
                    for ic in range(NT):
                        lhsT = attn_t[:, ic, jb * P : (jb + 1) * P]
                        nc.tensor.matmul(
                            ops[:, 0:512], lhsT, lpo_t[:, ic, 0:512],
                            start=False, stop=(ic == NT - 1),
                        )
                        nc.tensor.matmul(
                            ops[:, 512:D], lhsT, lpo_t[:, ic, 512:D],
                            start=False, stop=(ic == NT - 1),
                        )
                    outsb = outsb_pool.tile([P, D], F32, tag="outsb")
                    nc.vector.scalar_tensor_tensor(
                        outsb[:], ops[:], 1.0, bo_bc[:], op0=Mult, op1=Add
                    )
                    nc.sync.dma_start(out[jb * P : (jb + 1) * P, :], outsb[:])

            bcctx.close()

    nc.compile()
    return nc


def _build_sharded(nc):
    """Cache a jitted sharded executable so repeat calls skip retracing."""
    import jax
    import concourse.mybir as mybir
    from jax.sharding import Mesh, PartitionSpec
    from jax.experimental.shard_map import shard_map
    from concourse.bass2jax import (
        _bass_exec_p,
        install_neuronx_cc_hook,
        partition_id_tensor,
    )

    install_neuronx_cc_hook()
    partition_name = nc.partition_id_tensor.name if nc.partition_id_tensor else None
    in_names, out_names, out_avals, zero_outs = [], [], [], []
    for alloc in nc.m.functions[0].allocations:
        if not isinstance(alloc, mybir.MemoryLocationSet):
            continue
        name = alloc.memorylocations[0].name
        if alloc.kind == "ExternalInput":
            if name != partition_name:
                in_names.append(name)
        elif alloc.kind == "ExternalOutput":
            shape = tuple(alloc.tensor_shape)
            dtype = mybir.dt.np(alloc.dtype)
            out_names.append(name)
            out_avals.append(jax.core.ShapedArray(shape, dtype))
            zero_outs.append(np.zeros(shape, dtype))
    n_params = len(in_names)
    n_outs = len(out_avals)
    all_in_names = list(in_names) + list(out_names)
    if partition_name is not None:
        all_in_names.append(partition_name)
    donate = tuple(range(n_params, n_params + n_outs))

    def _body(*args):
        operands = list(args)
        if partition_name is not None:
            operands.append(partition_id_tensor())
        return tuple(
            _bass_exec_p.bind(
                *operands,
                out_avals=tuple(out_avals),
                in_names=tuple(all_in_names),
                out_names=tuple(out_names),
                lowering_input_output_aliases=(),
                sim_require_finite=True,
                sim_require_nnan=True,
                nc=nc,
            )
        )

    devices = jax.devices()[:B]
    mesh = Mesh(np.asarray(devices), ("core",))
    sharding = jax.sharding.NamedSharding(mesh, PartitionSpec("core"))
    sharded = jax.jit(
        shard_map(
            _body,
            mesh=mesh,
            in_specs=(PartitionSpec("core"),) * (n_params + n_outs),
            out_specs=(PartitionSpec("core"),) * n_outs,
            check_rep=False,
        ),
        donate_argnums=donate,
        keep_unused=True,
    )

    import jax.numpy as jnp

    zero_shapes = tuple((B * z.shape[0], *z.shape[1:]) for z in zero_outs)
    zero_dtypes = tuple(z.dtype for z in zero_outs)

    @jax.jit
    def _make_zeros():
        return tuple(jnp.zeros(s, d) for s, d in zip(zero_shapes, zero_dtypes))

    def device_zeros():
        return jax.device_put(_make_zeros(), [sharding] * len(zero_shapes))

    return {
        "sharded": sharded,
        "in_names": in_names,
        "out_names": out_names,
        "zero_outs": zero_outs,
        "out_avals": out_avals,
        "sharding": sharding,
        "device_zeros": device_zeros,
    }


def _prep_inputs(vision_features, language_features, Wv, bv, Wl, bl, Wo, bo):
    import ml_dtypes

    wvT = np.ascontiguousarray(np.asarray(Wv, dtype=np.float32).T)
    wlT = np.ascontiguousarray(np.asarray(Wl, dtype=np.float32).T)
    woT = np.ascontiguousarray(
        np.asarray(Wo, dtype=np.float32).T.astype(ml_dtypes.bfloat16)
    )
    bv = np.asarray(bv, dtype=np.float32)
    bl = np.asarray(bl, dtype=np.float32)
    bo = np.asarray(bo, dtype=np.float32)
    vision_features = np.asarray(vision_features, dtype=np.float32)
    language_features = np.asarray(language_features, dtype=np.float32)

    in_maps = []
    for b in range(B):
        in_maps.append(
            {
                "xvT": np.ascontiguousarray(vision_features[b].T),
                "xlT": np.ascontiguousarray(language_features[b].T),
                "wvT": wvT,
                "wlT": wlT,
                "woT": woT,
                "bv": bv,
                "bl": bl,
                "bo": bo,
            }
        )
    return in_maps


def kernel(
    vision_features, language_features, Wv, bv, Wl, bl, Wo, bo
) -> np.ndarray:
    from concourse.bass_utils import run_bass_kernel_spmd

    nc = _CACHE.get("nc")
    if nc is None:
        nc = _build_nc()
        _CACHE["nc"] = nc

    in_maps = _prep_inputs(
        vision_features, language_features, Wv, bv, Wl, bl, Wo, bo
    )

    try:
        ex = _CACHE.get("ex")
        if ex is None:
            ex = _build_sharded(nc)
            _CACHE["ex"] = ex
        concat_in = [
            np.concatenate([m[n] for m in in_maps], axis=0)
            for n in ex["in_names"]
        ]
        out_arrs = ex["sharded"](*concat_in, *ex["device_zeros"]())
        i = ex["out_names"].index("out")
        full = np.asarray(out_arrs[i]).reshape(B, *ex["out_avals"][i].shape)
        return full.astype(np.float32)
    except Exception:
        res = run_bass_kernel_spmd(nc, in_maps, list(range(B)))
        return np.stack([res.results[b]["out"] for b in range(B)]).astype(np.float32)
